# revision 30
# baseline (speedup 1.0000x reference)
"""Trainium2 Bass kernel for nn_BaconAdditionReasoner (histogram_binning).

Math (per batch row b):
    P1 = soft_perm(W1), P2 = soft_perm(W2)          (host, 10x10)
    u = log(1 - p1 @ P1.T), v = log(1 - p2 @ P2.T)  (host prep, f16 upload)
    log1m[i,j] = log(1 - min(l1_i, l2_j)) == max(u_i, v_j)
    logprod[k] = sum_{i+j=k} max(u_i, v_j)
              = sum_{i+j=k} u_i  +  sum_{i+j=k} relu(v_j - u_i)
    e_k = exp(logprod_k)                            (device output)
    y_k = (1 - e_k) / sum_j (1 - e_j)               (host normalize)

Device dataflow (pure data parallel over 8 cores, 32768 rows/core):
  Layout: features on partitions, batch on the free dim, 4 elements packed
  per column on 32-aligned 20-row bands (batch row = 4*col + band; see
  _build_uv).  Per supertile of width w: 4 D matmuls (20->110 pair diffs,
  one per band, into 4 single-bank psum quarters), relu psum->sbuf f16
  (ACT: q0 + q1[0:AR1]; DVE: the rest), 4 col-tiled A matmuls (110->32
  anti-diagonal bin sums), Exp on ACT (lag-1), one out-DMA per supertile.

  Cost-model facts driving the design (instruction_cost_v2.rs + hw_specs):
  - matmul: out_free_cols x 0.4167ns (f16), independent of row count
    -> PE floor = 8 passes x 8192 cols = 27.3us; PE is the bottleneck and
    ACT (1662ns/supertile) / DVE (1658) are co-saturated with it (1707),
    so no engine has slack to offload PE work
  - head is ~3.2us: 0.67us framework preamble (sem-range clear+barrier) +
    25 SEQ + 625 HWDGE + 650 DGE + transfer + 900 sem-prop for the first
    chunk; first supertile 384 wide matches chunk-1's serialized arrival
  - tail is ~3.8us: last A -> Exp(15)/Exp(16) -> ONE merged out-DMA for
    the last two supertiles (separate DMAs collide on HWDGE; a second DMA
    would queue >=625ns behind it on the in-order SP SEQ)
  - variable-width supertiles [384, 512x14, 384, 256]: big enough that
    the penultimate exp+DMA flush before PE finishes, small final exp
  - matmul psum outputs must stay inside ONE 2KB bank (512 f32 cols)
  - GPSIMD/Pool cannot touch PSUM or run TensorTensor (BIR verifier,
    walrus codegen) so the idle Pool engine cannot help; fp8/DoubleRow
    (0.5 cyc/col) breaks accuracy (6% element error into exp); PSUM
    cannot be DMA'd to DRAM, so Exp doubles as the forced psum->sbuf copy
  - f16 intermediates: measured end-to-end rel err 2.5e-3 vs 2e-2 gate
"""

import numpy as np

# ---------------------------------------------------------------- constants
B = 262144
NCORES = 8
BC = B // NCORES            # 32768 rows per core
CH = 4                      # band count (32-aligned partition bands)
NCOLS = BC // CH            # 8192 sbuf columns in uv / ep
KD = 110                    # pair rows (100) + passthrough -u rows (10)

# supertile widths (sum = NCOLS); sizes chosen so chunk i's DMA completes
# before its first consumer needs it (first ST 384 ~= chunk-1 arrival), and
# a small final ST shrinks the post-PE exp->DMA drain
WIDTHS = [384] + [512] * 14 + [384, 256]
# input chunk widths (sum = NCOLS; chunk 0 rides with wk16 in the head DMA);
# boundaries align with supertile boundaries
CHUNKS = [384, 512, 512, 512, 512, 1536, 2048, 2176]

# wk16 column layout
WD0, WD1 = 0, 110           # D weights  [20, 110]
WA0, WA1 = 110, 142         # A weights  [110, 32]
WK16C = 142
AR1 = 304                   # cols of dp quarter 1 relu'd on ACT (512-wide ST)


def _soft_perm_np(W: np.ndarray) -> np.ndarray:
    W = W.astype(np.float32)
    lo = W.min(axis=1, keepdims=True)
    hi = W.max(axis=1, keepdims=True)
    Wn = (W - lo) / (hi - lo + np.float32(1e-8))
    return Wn / (Wn.sum(axis=1, keepdims=True) + np.float32(1e-8))


def _build_wk16() -> np.ndarray:
    wk = np.zeros((128, WK16C), dtype=np.float16)
    # --- D [20, 110]: col 10i+j gets v_j - u_i ; col 100+e passes -u_e.
    #     Replicated at each 32-row band (fmap and weights share a base).
    d = np.zeros((20, 110), dtype=np.float16)
    for i in range(10):
        for j in range(10):
            d[i, 10 * i + j] = -1.0
            d[10 + j, 10 * i + j] = 1.0
    for e in range(10):
        d[e, 100 + e] = -1.0
    for q in range(4):
        wk[32 * q : 32 * q + 20, WD0:WD1] = d
    # --- A [110, 32]: pair row 10i+j -> +1 at k=i+j ; row 100+e -> -1 for
    #     k in [e, e+9] (those rows hold -u, so -1 gives +u)
    for i in range(10):
        for j in range(10):
            wk[10 * i + j, WA0 + i + j] = 1.0
    for e in range(10):
        wk[100 + e, WA0 + e : WA0 + e + 10] = -1.0
    return wk


def _build_uv(uc: np.ndarray, vc: np.ndarray) -> np.ndarray:
    """u,v [BC,10] f32 -> uv [128, NCOLS] f16: band q rows 32q+(0..9)=u,
    32q+(10..19)=v; col c = batch row 4c + q ... element (q, c) = batch row
    CH*c + q?  We use: batch row index = c * CH + q  (column-major over
    bands) so that unpack is a simple reshape."""
    u = uc.reshape(NCOLS, CH, 10).transpose(1, 2, 0)   # [CH, 10, NCOLS]
    v = vc.reshape(NCOLS, CH, 10).transpose(1, 2, 0)
    out = np.zeros((128, NCOLS), dtype=np.float16)
    for q in range(CH):
        out[32 * q : 32 * q + 10] = u[q]
        out[32 * q + 10 : 32 * q + 20] = v[q]
    return out


def _unpack_y(yraw: np.ndarray) -> np.ndarray:
    """yraw [128, NCOLS] f16 (band q rows 32q+k, k<19: e_k = exp(logprod_k))
    -> y [BC, 19] f32 via y = (1-e) / sum_k(1-e_k)."""
    t = yraw.astype(np.float32).reshape(4, 32, NCOLS)  # [q, 32, c]
    t = t[:, :19, :].transpose(2, 0, 1).reshape(BC, 19)
    t = 1.0 - t
    return t / t.sum(axis=1, keepdims=True)


ROLES = {}


def _rec(role, obj):
    try:
        ROLES[obj.ins.name] = role
    except Exception:
        pass
    return obj


def build_bass():
    import concourse.bacc as bacc
    import concourse.tile as tile
    from concourse import mybir

    f32 = mybir.dt.float32
    f16 = mybir.dt.float16
    AF = mybir.ActivationFunctionType
    ALU = mybir.AluOpType

    nc = bacc.Bacc("TRN2", target_bir_lowering=False)

    # head = wk16 columns + the first (tiny) uv chunk in ONE DMA: the first
    # D matmul gates on this DMA's full latency chain, so keep it small.
    hd_d = nc.dram_tensor(
        "head", [128, WK16C + CHUNKS[0]], f16, kind="ExternalInput"
    )
    uv_d = nc.dram_tensor(
        "uvp", [128, NCOLS - CHUNKS[0]], f16, kind="ExternalInput"
    )
    y_d = nc.dram_tensor("yraw", [128, NCOLS], f16, kind="ExternalOutput")

    NST = len(WIDTHS)
    offs = np.concatenate([[0], np.cumsum(WIDTHS)]).astype(int)

    with tile.TileContext(nc) as tc:
        with (
            tc.tile_pool(name="singles", bufs=1) as singles,
            tc.tile_pool(name="kt", bufs=3) as kt_p,
            tc.tile_pool(name="kt2", bufs=3) as kt2_p,
            tc.tile_pool(name="ep", bufs=4) as ep_p,
            tc.tile_pool(name="psD", bufs=6, space="PSUM") as psD,
            tc.tile_pool(name="psZ", bufs=2, space="PSUM") as psZ,
        ):
            head = singles.tile([128, WK16C + CHUNKS[0]], f16)
            # rows 116-127 are structurally zero; 116 rows is the minimum
            nc.sync.dma_start(head[0:116, :], hd_d[0:116, :])
            wk = head  # wk16 columns live at head[:, 0:WK16C]

            packs = [(0, CHUNKS[0], None)]
            c0 = CHUNKS[0]
            for w in CHUNKS[1:]:
                p = singles.tile([128, w], f16, name=f"pk{c0}")
                nc.sync.dma_start(
                    p[0:116, :],
                    uv_d[0:116, c0 - CHUNKS[0] : c0 - CHUNKS[0] + w],
                )
                packs.append((c0, w, p))
                c0 += w

            def uv_slice(col0, w, r0, r1):
                if col0 < CHUNKS[0]:
                    assert col0 + w <= CHUNKS[0]
                    return head[r0:r1, WK16C + col0 : WK16C + col0 + w]
                for cc0, cw, p in packs:
                    if p is not None and cc0 <= col0 and col0 + w <= cc0 + cw:
                        return p[r0:r1, col0 - cc0 : col0 - cc0 + w]
                raise AssertionError((col0, w))

            # Software pipeline, one iteration per supertile `it`:
            #   PE:  D(it)x4, A(it-1)x4
            #   ACT: Exp(it-2), relu q0+q1[0:a1](it)
            #   DVE: relu rest(it)
            #   SP:  DMA of ep(it-2) straight from the Exp output tile
            kts, eps = {}, {}
            for it in range(NST + 1):
                # ---- D(it) + relu(it)
                if it < NST:
                    off, w = int(offs[it]), WIDTHS[it]
                    ktq = [
                        kt_p.tile([KD, 512], f16, name=f"ktq{h}")
                        for h in range(2)
                    ] + [
                        kt2_p.tile([KD, 512], f16, name=f"ktq{h}")
                        for h in range(2, 4)
                    ]
                    # one 6-bank ring, allocation order (4*it+g) mod 6:
                    # q2/q3 land on banks cleared by FAST ACT relus one
                    # iteration prior; q0/q1 on banks cleared by DVE relus
                    # two iterations prior -- strictly more WAR slack than
                    # any static double-buffering split.
                    dps = [
                        psD.tile([KD, 512], f32, name="dp")
                        for _ in range(4)
                    ]
                    for g in range(4):
                        _rec(f"D{g}({it})", nc.tensor.matmul(
                            dps[g][0:KD, 0:w],
                            wk[32 * g : 32 * g + 20, WD0:WD1],
                            uv_slice(off, w, 32 * g, 32 * g + 20),
                            start=True, stop=True,
                            tile_position=(32 * g, 0),
                        ))
                    # relu: ACT covers q0 + q1[0:a1], DVE the rest (AR1
                    # tuned for w=512; last ST gives ACT all of q1)
                    a1 = (AR1 * w) // 512
                    if it == NST - 1:
                        a1 = w
                    _rec(f"reluA0({it})", nc.scalar.activation(
                        ktq[0][0:KD, 0:w], dps[0][0:KD, 0:w], AF.Relu
                    ))
                    _rec(f"reluA1({it})", nc.scalar.activation(
                        ktq[1][0:KD, 0:a1], dps[1][0:KD, 0:a1], AF.Relu
                    ))
                    # DVE issue order matches A's consumption order
                    # (0,2,3,1): q2, q3 first, the q1 remainder last
                    _rec(f"reluV2({it})", nc.vector.tensor_scalar(
                        ktq[2][0:KD, 0:w], dps[2][0:KD, 0:w],
                        0.0, None, op0=ALU.max,
                    ))
                    _rec(f"reluV3({it})", nc.vector.tensor_scalar(
                        ktq[3][0:KD, 0:w], dps[3][0:KD, 0:w],
                        0.0, None, op0=ALU.max,
                    ))
                    if a1 < w:
                        _rec(f"reluV1({it})", nc.vector.tensor_scalar(
                            ktq[1][0:KD, a1:w], dps[1][0:KD, a1:w],
                            0.0, None, op0=ALU.max,
                        ))
                    kts[it] = ktq
                # ---- A(it-1): 4 col-tiled f16 matmuls, band q <- quarter q
                sA = it - 1
                if 0 <= sA < NST:
                    wA = WIDTHS[sA]
                    ktq = kts.pop(sA)
                    zz = psZ.tile([128, 512], f32)
                    ag = (2, 0, 3, 1) if sA == NST - 1 else (0, 2, 3, 1)
                    for g in ag:
                        _rec(f"A{g}({sA})", nc.tensor.matmul(
                            zz[32 * g : 32 * g + 32, 0:wA],
                            wk[0:KD, WA0:WA1],
                            ktq[g][0:KD, 0:wA],
                            start=True, stop=True,
                            tile_position=(0, 32 * g),
                        ))
                    eps[sA] = zz
                # ---- Exp(it-1), then DMA the e values straight out.
                # The last two supertiles share one ep tile and ONE final
                # DMA so the tail has a single HWDGE+DGE+sem chain.
                sE = it - 1
                if 0 <= sE < NST:
                    wE = WIDTHS[sE]
                    zz = eps.pop(sE)
                    if sE == NST - 2:
                        ep_last = ep_p.tile([128, 1024], f16, name="eplast")
                        _rec(f"Exp({sE})", nc.scalar.activation(
                            ep_last[0:128, 0:wE], zz[0:128, 0:wE], AF.Exp
                        ))
                    elif sE == NST - 1:
                        w2 = WIDTHS[NST - 2]
                        _rec(f"Exp({sE})", nc.scalar.activation(
                            ep_last[0:128, w2 : w2 + wE], zz[0:128, 0:wE],
                            AF.Exp
                        ))
                        oc0 = int(offs[NST - 2])
                        nc.sync.dma_start(
                            y_d[0:116, oc0 : oc0 + w2 + wE],
                            ep_last[0:116, 0 : w2 + wE],
                        )
                    else:
                        ep = ep_p.tile([128, 512], f16)
                        _rec(f"Exp({sE})", nc.scalar.activation(
                            ep[0:128, 0:wE], zz[0:128, 0:wE], AF.Exp
                        ))
                        oc0 = int(offs[sE])
                        nc.sync.dma_start(
                            y_d[0:116, oc0 : oc0 + wE], ep[0:116, 0:wE]
                        )
    nc.compile()
    return nc


_NC_CACHE = None


def kernel(p1, p2, W1, W2):
    global _NC_CACHE
    from concourse.bass_utils import run_bass_kernel_spmd

    P1n = _soft_perm_np(np.asarray(W1))
    P2n = _soft_perm_np(np.asarray(W2))
    wk16 = _build_wk16()
    p1 = np.asarray(p1, dtype=np.float32)
    p2 = np.asarray(p2, dtype=np.float32)
    u = np.log1p(-(p1 @ P1n.T)).astype(np.float32)
    v = np.log1p(-(p2 @ P2n.T)).astype(np.float32)

    in_maps = []
    c0 = CHUNKS[0]
    for c in range(NCORES):
        sl = slice(c * BC, (c + 1) * BC)
        uvp = _build_uv(u[sl], v[sl])
        head = np.concatenate([wk16, uvp[:, :c0]], axis=1)
        in_maps.append(
            {"head": np.ascontiguousarray(head), "uvp": uvp[:, c0:].copy()}
        )

    if _NC_CACHE is None:
        _NC_CACHE = build_bass()
    res = run_bass_kernel_spmd(_NC_CACHE, in_maps, core_ids=list(range(NCORES)))
    out = np.concatenate(
        [_unpack_y(res.results[c]["yraw"]) for c in range(NCORES)], axis=0
    )
    return out
